# revision 23
# baseline (speedup 1.0000x reference)
"""Trainium2 Bass kernel: 12-head attention with relative position bias.

Reference computation (B=64, N=197, DIM=768, H=12, HD=64):
    qkv = x @ Wqkv.T + [q_bias, 0, v_bias]
    q, k, v = split(qkv); q *= HD**-0.5
    attn = softmax(q @ k.T + rel_table[rel_index].T)   # bias per head
    y = (attn @ v).reshape @ Wproj.T + bproj

Strategy: pure data-parallel over batch (8 batches per NeuronCore x 8 cores,
no collectives), bf16 matmuls with fp32 PSUM. Attention is computed
transposed (attnT = k q^T) so attn @ v needs no transpose; softmax has no
max-subtraction (|logits| < 3); per-query sums via ones-vector matmuls;
normalization applied post-AV via a DRAM-staged partition broadcast of
reciprocal sums. Exact host-side algebra folds remove device work:
  - exp(rel_bias) precomputed on host, loaded as one dense tile
  - v_bias and bproj leave the device entirely:
      y_device = (attn @ (x@Wv)) / sums @ Wp.T
      y = y_device + (bproj + v_bias @ Wp.T)     [host, exact]
The kernel is software-pipelined per batch slot s:
  norm-apply(s-1) on gpsimd | QK+exp+mult(s+1) | proj(s-2) interleaved
  with AV+sums(s) | norm chain(s) on sync/vector.
Leftover v-projections (batches 6,7) fill slots 0/1 so the PE stays busy
and the HAM clock gate keeps the PE at 2.4 GHz.
"""
import sys
sys.path.insert(0, '/opt/trn_rl_repo')
import itertools
import numpy as np
import ml_dtypes

import concourse.bass as bass
import concourse.mybir as mybir
from concourse import tile as _tile
from concourse.tile import TileContext, add_dep_helper
from concourse.vector_clock import ScopedClock
from concourse.bass_utils import run_bass_kernel_spmd

# ---------------------------------------------------------------------------
# Patches for this toolchain's one-sync-wait-per-instruction limit.
# ---------------------------------------------------------------------------
_counter = itertools.count()


def _drain_and_barrier_split(self, tick_clock, wait_clock):
    vc = tick_clock.global_clock
    for proc in range(len(vc)):
        t = vc[proc]
        if t > 0:
            sc = ScopedClock()
            sc.require_at_least(None, proc, t)
            nop_inst = self.nc.sync.nop(nofuse=True, hint="drain_split")
            wait_clock.add_sem_waits(nop_inst.ins, sc)
    self.nc.sync.drain()
    self.nc.all_engine_barrier()
    popped = self.nc._tile_sem_poison_stack.pop()
    assert popped is self._sem_poison
    self.nc.clear_and_free_semaphores(list(self.sems.allocated().values()))
    self.nc.all_engine_barrier()


_tile.TileContext._drain_and_barrier = _drain_and_barrier_split

_RealTileClockWait = _tile.TileClockWait
if getattr(_RealTileClockWait, "_is_split_wrapper", False):  # re-import safety
    _RealTileClockWait = _RealTileClockWait._real


def _split_excess_waits(ordered):
    for bb_name, insts in ordered.items():
        out = []
        changed = False
        for inst in insts:
            si = inst.sync_info
            waits = list(si.on_wait) if si is not None and si.on_wait else []
            if len(waits) > 1:
                changed = True
                for w in waits[:-1]:
                    nop = mybir.InstNoOp(
                        name=f"waitsplit_{next(_counter)}", engine=inst.engine)
                    nop.sync_info = mybir.SyncInfo(on_wait=[w], on_update=[])
                    nop.bass_nofuse = True
                    out.append(nop)
                inst.sync_info = mybir.SyncInfo(
                    on_wait=[waits[-1]],
                    on_update=list(si.on_update) if si.on_update else [])
            out.append(inst)
        if changed:
            insts[:] = out


class _TileClockWaitSplit:
    _is_split_wrapper = True
    _real = _RealTileClockWait

    def __init__(self, *args, **kwargs):
        self._inner = _RealTileClockWait(*args, **kwargs)
        self._ordered = args[1] if len(args) > 1 else kwargs["ordered_instructions_by_block"]

    def __getattr__(self, k):
        return getattr(self._inner, k)

    def assign_waits(self, bb_name):
        r = self._inner.assign_waits(bb_name)
        _split_excess_waits(self._ordered)
        return r


_tile.TileClockWait = _TileClockWaitSplit

# ---------------------------------------------------------------------------
B, N, DIM, H, HD = 64, 197, 768, 12, 64
NCORES = 8
BL = B // NCORES            # 8 batches per core
NQ0, NQ1 = 127, 70          # keys split: keys 0..126 | keys 127..196
F32 = mybir.dt.float32
BF16 = mybir.dt.bfloat16
AF = mybir.ActivationFunctionType
ALU = mybir.AluOpType

_graph_cache = {}


def _build_graph():
    nc = bass.Bass()
    xT = nc.declare_dram_parameter("xT", [6, 128, BL * N], BF16, isOutput=False)
    wqkvT = nc.declare_dram_parameter("wqkvT", [DIM, 3 * DIM], BF16, isOutput=False)
    wprojT = nc.declare_dram_parameter("wprojT", [6, 128, DIM], BF16, isOutput=False)
    qkbias = nc.declare_dram_parameter("qkbias", [128, 12], F32, isOutput=False)
    expbT = nc.declare_dram_parameter("expbT", [NQ0, H * 2 * N], BF16, isOutput=False)
    onesb = nc.declare_dram_parameter("onesb", [128, H], BF16, isOutput=False)
    yT = nc.declare_dram_parameter("yT", [BL, DIM, N], BF16, isOutput=True)

    rstage = nc.dram_tensor("recip_stage", [BL, 2432], BF16)
    rstage_s = nc.dram_tensor("sums_stage", [BL, 2432], BF16)

    with nc.allow_low_precision(reason="bf16 compute validated: rel_err 4e-3 vs 2e-2 gate"), \
         TileContext(nc) as tc:
        with tc.tile_pool(name="const", bufs=1) as cpool, \
             tc.tile_pool(name="small", bufs=3) as spool:

            wqq = [cpool.tile([128, DIM], BF16, tag=f"wqq{c}", name=f"wqq{c}") for c in range(6)]
            wqk = [cpool.tile([128, DIM], BF16, tag=f"wqk{c}", name=f"wqk{c}") for c in range(6)]
            wqv = [cpool.tile([128, DIM], BF16, tag=f"wqv{c}", name=f"wqv{c}") for c in range(6)]
            wp = [cpool.tile([128, DIM], BF16, tag=f"wp{c}", name=f"wp{c}") for c in range(6)]
            qkb = cpool.tile([128, 12], F32, tag="qkb")
            ons = cpool.tile([128, H], BF16, tag="ons")
            expb = cpool.tile([NQ0, H * 2 * N], BF16, tag="expb")

            _xpool_cm = tc.tile_pool(name="xp", bufs=1)
            xpool = _xpool_cm.__enter__()
            xall = [xpool.tile([128, BL * N], BF16, tag=f"x{c}", name=f"x{c}") for c in range(6)]

            # ---- load schedule ------------------------------------------
            nc.sync.dma_start(out=qkb[:], in_=qkbias[:])
            nc.scalar.dma_start(out=ons[:], in_=onesb[:])
            # one DMA per tile (same-tile splits serialize in Tile);
            # x + wq-q are the critical set for the first matmuls, spread
            # over all three issuing rows (sync/scalar/gpsimd ~90GB/s each)
            engs = [nc.sync, nc.scalar, nc.gpsimd]
            _crit_loads = []
            for c in range(6):
                _crit_loads.append(engs[c % 2].dma_start(out=xall[c][:], in_=xT[c]))
            for c in range(6):
                _crit_loads.append(nc.gpsimd.dma_start(
                    out=wqq[c][:],
                    in_=bass.AP(wqkvT, c * 128 * 3 * DIM, [[3 * DIM, 128], [1, DIM]])))
            for c in range(6):
                engs[c % 2].dma_start(
                    out=wqk[c][:],
                    in_=bass.AP(wqkvT, c * 128 * 3 * DIM + DIM, [[3 * DIM, 128], [1, DIM]]))
            for c in range(6):
                engs[c % 2].dma_start(
                    out=wqv[c][:],
                    in_=bass.AP(wqkvT, c * 128 * 3 * DIM + 2 * DIM, [[3 * DIM, 128], [1, DIM]]))
            nc.gpsimd.dma_start(out=expb[:], in_=expbT[:])
            for c in range(6):
                nc.gpsimd.dma_start(out=wp[c][:], in_=wprojT[c])

            # ---- qk projection: qkTm[m] = [128, BL*197] bf16 ------------
            qkTm = [cpool.tile([128, BL * N], BF16, tag=f"qk{m}", name=f"qkTm{m}") for m in range(12)]
            with tc.tile_pool(name="ps_qk", bufs=8, space="PSUM") as pqk:
                for m in range(12):
                    pss = [pqk.tile([128, 2 * N], F32, tag="qkps", name=f"qkps{m}_{_}") for _ in range(4)]
                    wsrc = wqq if m < 6 else wqk
                    mc = m % 6
                    for c in range(6):
                        for bp in range(4):
                            nc.tensor.matmul(
                                pss[bp][:],
                                wsrc[c][:, 128 * mc:128 * (mc + 1)],
                                xall[c][:, bp * 2 * N:(bp + 1) * 2 * N],
                                start=(c == 0), stop=(c == 5))
                    sc = 0.125 if m < 6 else 1.0
                    for bp in range(4):
                        nc.vector.tensor_scalar(
                            out=qkTm[m][:, bp * 2 * N:(bp + 1) * 2 * N],
                            in0=pss[bp][:], scalar1=sc,
                            scalar2=qkb[:, m:m + 1], op0=ALU.mult, op1=ALU.add)

            # ---- v projection (no v_bias: folded to host) ---------------
            v_sb = [[cpool.tile([NQ0 if nch == 0 else NQ1, DIM], BF16,
                                tag=f"v{b}_{nch}", name=f"v{b}_{nch}") for nch in range(2)] for b in range(BL)]

            def emit_vproj(b, psum_pool, tag, shape):
                for nch in range(2):
                    nn_, nb = (NQ0, 0) if nch == 0 else (NQ1, NQ0)
                    ps = [psum_pool.tile(shape, F32, tag=tag,
                                         name=f"vps{b}_{nch}_{_}") for _ in range(2)]
                    for c in range(6):
                        for fh in range(2):
                            nc.tensor.matmul(
                                ps[fh][0:nn_, 0:384],
                                xall[c][:, b * N + nb:b * N + nb + nn_],
                                wqv[c][:, 384 * fh:384 * (fh + 1)],
                                start=(c == 0), stop=(c == 5))
                    for fh in range(2):
                        if fh == 0:
                            nc.scalar.activation(
                                out=v_sb[b][nch][0:nn_, 0:384],
                                in_=ps[0][0:nn_, 0:384], func=AF.Copy)
                        else:
                            nc.vector.tensor_copy(
                                out=v_sb[b][nch][0:nn_, 384:768],
                                in_=ps[1][0:nn_, 0:384])

            _pv_cm = tc.tile_pool(name="ps_v", bufs=4, space="PSUM")
            pv = _pv_cm.__enter__()
            for b in range(6):
                emit_vproj(b, pv, "vps", [NQ0, 2 * N])
            _pv_cm.__exit__(None, None, None)

            _apool_cm = tc.tile_pool(name="ap", bufs=1)
            apool = _apool_cm.__enter__()

            outT = [[apool.tile([128, N], BF16, tag=f"o{cc}", name=f"outT{b}_{cc}", bufs=3) for cc in range(6)]
                    for b in range(BL)]
            ups = {}
            rbcs = {}
            ehms = {}

            with tc.tile_pool(name="ps_at", bufs=2, space="PSUM") as pat, \
                 tc.tile_pool(name="ps_av", bufs=1, space="PSUM") as pav, \
                 tc.tile_pool(name="ps_pj", bufs=1, space="PSUM") as ppjp, \
                 tc.tile_pool(name="ps_ss", bufs=2, space="PSUM") as pssp:
                ehms = {}

                def emit_qk_pair(b, hp):
                    mq, mk = hp, 6 + hp
                    psh = pat.tile([NQ0, 1024], F32, tag="atps",
                                   name=f"atps{b}_{hp}")
                    ehp = apool.tile([NQ0, 4 * N], BF16, tag="ehs",
                                     name=f"eh{b}_{hp}", bufs=3)
                    for hh in range(2):
                        rb = hh * 64
                        q0 = hh * 512
                        nc.tensor.matmul(
                            psh[0:NQ0, q0:q0 + N],
                            qkTm[mk][rb:rb + 64, b * N:b * N + NQ0],
                            qkTm[mq][rb:rb + 64, b * N:b * N + N],
                            start=True, stop=True)
                        nc.tensor.matmul(
                            psh[0:NQ1, q0 + N:q0 + 2 * N],
                            qkTm[mk][rb:rb + 64, b * N + NQ0:b * N + N],
                            qkTm[mq][rb:rb + 64, b * N:b * N + N],
                            start=True, stop=True)
                    nc.scalar.activation(
                        out=bass.AP(ehp[:].tensor, 0, [[4 * N, NQ0], [2 * N, 2], [1, 2 * N]]),
                        in_=bass.AP(psh[:].tensor, 0, [[1024, NQ0], [512, 2], [1, 2 * N]]),
                        func=AF.Exp)
                    ehm = ehms[b][hp]
                    nc.vector.tensor_tensor(
                        out=ehm[:], in0=ehp[:],
                        in1=expb[0:NQ0, hp * 4 * N:(hp + 1) * 4 * N],
                        op=ALU.mult)

                for slot in range(BL + 1):
                    if slot == 0:
                        ehms[0] = [apool.tile([NQ0, 4 * N], BF16, tag=f"em{hp}",
                                              name=f"em0_{hp}", bufs=2) for hp in range(6)]
                        for hp in range(6):
                            emit_qk_pair(0, hp)
                    # ---- norm-apply for batch slot-1 (gpsimd) ----
                    bm1 = slot - 1
                    if 0 <= bm1 < BL:
                        rbc = rbcs.pop(bm1)
                        up = ups.pop(bm1)
                        for cc in range(6):
                            nc.gpsimd.tensor_tensor(
                                out=outT[bm1][cc][:], in0=up[cc][:],
                                in1=rbc[:, cc * N:(cc + 1) * N], op=ALU.mult)

                    b = slot if slot < BL else None
                    bnx = slot + 1 if slot + 1 < BL else None
                    bm2 = slot - 2
                    # proj schedule: lag 2 for b<=4, lag 1 for b>=5 (shorter
                    # pipeline drain); maps step index -> (batch, pair)
                    proj_steps = {}
                    if 2 <= slot <= 6:
                        for pr in range(3):
                            proj_steps[pr] = (slot - 2, pr)
                    if 6 <= slot <= 8:
                        for pr in range(3):
                            proj_steps[3 + pr] = (slot - 1, pr)
                    if bnx is not None:
                        ehms[bnx] = [apool.tile([NQ0, 4 * N], BF16, tag=f"em{hp}",
                                                name=f"em{bnx}_{hp}", bufs=2) for hp in range(6)]
                    if b is not None:
                        ehm_b = ehms.pop(b)
                        srow = apool.tile([1, 2432], BF16, tag="srow", bufs=3,
                                          name=f"srow{b}")
                        up = [apool.tile([128, N], BF16, tag=f"up{cc}",
                                         name=f"up{b}_{cc}", bufs=2) for cc in range(6)]
                        ups[b] = up

                    def emit_proj_pair(pb, pr):
                        ppx = ppjp.tile([128, 2 * N], F32, tag="pjps",
                                        name=f"pj{pb}_{pr}")
                        for half in range(2):
                            mp = 2 * pr + half
                            for c in range(6):
                                nc.tensor.matmul(
                                    ppx[:, half * N:(half + 1) * N],
                                    wp[c][:, 128 * mp:128 * (mp + 1)],
                                    outT[pb][c][:], start=(c == 0), stop=(c == 5))
                        ysb = spool.tile([128, 2 * N], BF16, tag="ysb", name=f"ysb{pb}_{pr}")
                        nc.scalar.activation(out=ysb[:], in_=ppx[:], func=AF.Copy)
                        nc.sync.dma_start(
                            out=bass.AP(yT, pb * DIM * N + 2 * pr * 128 * N,
                                        [[N, 128], [128 * N, 2], [1, N]]),
                            in_=ysb[:])

                    def emit_av(hp):
                        pp = pav.tile([128, N], F32, tag="avps", name=f"avps{b}_{hp}")
                        for hh in range(2):
                            h = 2 * hp + hh
                            rb = hh * 64
                            e0 = hh * 2 * N
                            nc.tensor.matmul(pp[rb:rb + 64, :],
                                             v_sb[b][0][:, h * 64:(h + 1) * 64],
                                             ehm_b[hp][0:NQ0, e0:e0 + N],
                                             start=True, stop=False,
                                             tile_position=(0, rb))
                            nc.tensor.matmul(pp[rb:rb + 64, :],
                                             v_sb[b][1][:, h * 64:(h + 1) * 64],
                                             ehm_b[hp][0:NQ1, e0 + N:e0 + 2 * N],
                                             start=False, stop=True,
                                             tile_position=(0, rb))
                        nc.vector.tensor_copy(out=up[hp][:], in_=pp[:])

                    def emit_sums(hp):
                        pss = pssp.tile([1, 2 * N], F32, tag="smps", name=f"smps{b}_{hp}")
                        for hh in range(2):
                            e0 = hh * 2 * N
                            nc.tensor.matmul(pss[0:1, hh * N:(hh + 1) * N],
                                             ons[0:NQ0, 0:1],
                                             ehm_b[hp][0:NQ0, e0:e0 + N],
                                             start=True, stop=False)
                            nc.tensor.matmul(pss[0:1, hh * N:(hh + 1) * N],
                                             ons[0:NQ1, 0:1],
                                             ehm_b[hp][0:NQ1, e0 + N:e0 + 2 * N],
                                             start=False, stop=True)
                        if hp % 2 == 0:
                            nc.scalar.activation(
                                out=srow[0:1, hp * 2 * N:(hp + 1) * 2 * N],
                                in_=pss[0:1, :], func=AF.Copy)
                        else:
                            nc.vector.tensor_copy(
                                out=srow[0:1, hp * 2 * N:(hp + 1) * 2 * N], in_=pss[0:1, :])

                    # interleaved PE emission: QK pairs spaced by AV/proj;
                    # sums lag their AV by one step so the srow copy hides
                    for step in range(6):
                        if bnx is not None:
                            emit_qk_pair(bnx, step)
                        if slot in (0, 1) and step == 0:
                            emit_vproj(6 + slot, pat, "atps", [NQ0, 1024])
                        if b is not None:
                            emit_av(step)
                            if step > 0:
                                emit_sums(step - 1)
                        if step in proj_steps:
                            emit_proj_pair(*proj_steps[step])
                    if b is not None:
                        emit_sums(5)

                    # ---- normalization chain for batch slot ----
                    if b is not None:
                        swr = nc.sync.dma_start(out=rstage_s[b:b + 1, :], in_=srow[0:1, :])
                        s128 = apool.tile([128, 19], BF16, tag="s128", bufs=3)
                        srd = nc.sync.dma_start(
                            out=s128[:], in_=bass.AP(rstage_s, b * 2432, [[19, 128], [1, 19]]))
                        add_dep_helper(srd.ins, swr.ins, sync=True, reason="sums staging")
                        r128 = apool.tile([128, 19], BF16, tag="r128", bufs=3)
                        nc.vector.reciprocal(out=r128[:], in_=s128[:])
                        rwr = nc.sync.dma_start(
                            out=bass.AP(rstage, b * 2432, [[19, 128], [1, 19]]), in_=r128[:])
                        rbc = apool.tile([128, 6 * N], BF16, tag="rbc", bufs=3,
                                         name=f"rbc{b}")
                        rbcs[b] = rbc
                        rrd = nc.sync.dma_start(
                            out=bass.AP(rbc[:].tensor, 0, [[6 * N, 64], [N, 6], [1, N]]),
                            in_=bass.AP(rstage, b * 2432, [[0, 64], [2 * N, 6], [1, N]]))
                        rrd2 = nc.sync.dma_start(
                            out=bass.AP(rbc[:].tensor, 64 * 6 * N, [[6 * N, 64], [N, 6], [1, N]]),
                            in_=bass.AP(rstage, b * 2432 + N, [[0, 64], [2 * N, 6], [1, N]]))
                        add_dep_helper(rrd.ins, rwr.ins, sync=True, reason="recip staging")
                        add_dep_helper(rrd2.ins, rwr.ins, sync=True, reason="recip staging")
            _apool_cm.__exit__(None, None, None)
            _xpool_cm.__exit__(None, None, None)
    return nc


def _prep_inputs(x, Wqkv, q_bias, v_bias, rel_table, Wproj, bproj, rel_index):
    bf = ml_dtypes.bfloat16
    xs = np.asarray(x).astype(bf)                         # [B, N, DIM]
    xT = np.ascontiguousarray(xs.transpose(2, 0, 1))      # [DIM, B, N]
    wqkvT = np.ascontiguousarray(np.asarray(Wqkv).T).astype(bf)
    wprojT = np.ascontiguousarray(np.asarray(Wproj).T).astype(bf).reshape(6, 128, DIM)
    qs = np.concatenate([np.asarray(q_bias) * (HD ** -0.5), np.zeros(DIM, np.float32)])
    qkbias = np.ascontiguousarray(qs.reshape(12, 128).T).astype(np.float32)
    bias = np.asarray(rel_table)[np.asarray(rel_index)]   # [N(q), N(k), H]
    eb = np.exp(bias.transpose(2, 0, 1).astype(np.float32))  # [H, q, k]
    expbT = np.zeros((NQ0, H * 2 * N), dtype=np.float32)
    ebT = eb.transpose(0, 2, 1)                           # [H, k, q]
    for h in range(H):
        expbT[0:NQ0, h * 2 * N:h * 2 * N + N] = ebT[h, 0:NQ0, :]
        expbT[0:NQ1, h * 2 * N + N:(h + 1) * 2 * N] = ebT[h, NQ0:N, :]
    expbT = expbT.astype(bf)
    onesb = np.ones((128, H), dtype=bf)
    # exact host fold: y += bproj + v_bias @ Wproj.T
    ybias = (np.asarray(bproj) +
             np.asarray(v_bias).astype(np.float64) @ np.asarray(Wproj).astype(np.float64).T
             ).astype(np.float32)
    return xT, wqkvT, wprojT, qkbias, expbT, onesb, ybias


def run_sharded(inputs, trace=False):
    nc = _graph_cache.get("nc")
    if nc is None:
        nc = _build_graph()
        _graph_cache["nc"] = nc
    xT, wqkvT, wprojT, qkbias, expbT, onesb, ybias = _prep_inputs(**inputs)
    in_maps = []
    for i in range(NCORES):
        in_maps.append({
            "xT": np.ascontiguousarray(
                xT[:, i * BL:(i + 1) * BL, :].reshape(6, 128, BL * N)),
            "wqkvT": wqkvT, "wprojT": wprojT, "qkbias": qkbias,
            "expbT": expbT, "onesb": onesb,
        })
    res = run_bass_kernel_spmd(nc, in_maps, list(range(NCORES)), trace=trace)
    outs = []
    for i in range(NCORES):
        ytc = np.asarray(res.results[i]["yT"]).astype(np.float32)  # [BL, DIM, N]
        outs.append(ytc.transpose(0, 2, 1))             # [BL, N, DIM]
    y = np.concatenate(outs, axis=0).astype(np.float32) + ybias
    return y, res


def kernel(**inputs) -> np.ndarray:
    y, _ = run_sharded(inputs, trace=False)
    return y


# revision 24
# speedup vs baseline: 1.0022x; 1.0022x over previous
"""Trainium2 Bass kernel: 12-head attention with relative position bias.

Reference computation (B=64, N=197, DIM=768, H=12, HD=64):
    qkv = x @ Wqkv.T + [q_bias, 0, v_bias]
    q, k, v = split(qkv); q *= HD**-0.5
    attn = softmax(q @ k.T + rel_table[rel_index].T)   # bias per head
    y = (attn @ v).reshape @ Wproj.T + bproj

Strategy: pure data-parallel over batch (8 batches per NeuronCore x 8 cores,
no collectives), bf16 matmuls with fp32 PSUM. Attention is computed
transposed (attnT = k q^T) so attn @ v needs no transpose; softmax has no
max-subtraction (|logits| < 3); per-query sums via ones-vector matmuls;
normalization applied post-AV via a DRAM-staged partition broadcast of
reciprocal sums. Exact host-side algebra folds remove device work:
  - exp(rel_bias) precomputed on host, loaded as one dense tile
  - v_bias and bproj leave the device entirely:
      y_device = (attn @ (x@Wv)) / sums @ Wp.T
      y = y_device + (bproj + v_bias @ Wp.T)     [host, exact]
The kernel is software-pipelined per batch slot s:
  norm-apply(s-1) on gpsimd | QK+exp+mult(s+1) | proj(s-2) interleaved
  with AV+sums(s) | norm chain(s) on sync/vector.
Leftover v-projections (batches 6,7) fill slots 0/1 so the PE stays busy
and the HAM clock gate keeps the PE at 2.4 GHz.
"""
import sys
sys.path.insert(0, '/opt/trn_rl_repo')
import itertools
import numpy as np
import ml_dtypes

import concourse.bass as bass
import concourse.mybir as mybir
from concourse import tile as _tile
from concourse.tile import TileContext, add_dep_helper
from concourse.vector_clock import ScopedClock
from concourse.bass_utils import run_bass_kernel_spmd

# ---------------------------------------------------------------------------
# Patches for this toolchain's one-sync-wait-per-instruction limit.
# ---------------------------------------------------------------------------
_counter = itertools.count()


def _drain_and_barrier_split(self, tick_clock, wait_clock):
    vc = tick_clock.global_clock
    for proc in range(len(vc)):
        t = vc[proc]
        if t > 0:
            sc = ScopedClock()
            sc.require_at_least(None, proc, t)
            nop_inst = self.nc.sync.nop(nofuse=True, hint="drain_split")
            wait_clock.add_sem_waits(nop_inst.ins, sc)
    self.nc.sync.drain()
    self.nc.all_engine_barrier()
    popped = self.nc._tile_sem_poison_stack.pop()
    assert popped is self._sem_poison
    self.nc.clear_and_free_semaphores(list(self.sems.allocated().values()))
    self.nc.all_engine_barrier()


_tile.TileContext._drain_and_barrier = _drain_and_barrier_split

_RealTileClockWait = _tile.TileClockWait
if getattr(_RealTileClockWait, "_is_split_wrapper", False):  # re-import safety
    _RealTileClockWait = _RealTileClockWait._real


def _split_excess_waits(ordered):
    for bb_name, insts in ordered.items():
        out = []
        changed = False
        for inst in insts:
            si = inst.sync_info
            waits = list(si.on_wait) if si is not None and si.on_wait else []
            if len(waits) > 1:
                changed = True
                for w in waits[:-1]:
                    nop = mybir.InstNoOp(
                        name=f"waitsplit_{next(_counter)}", engine=inst.engine)
                    nop.sync_info = mybir.SyncInfo(on_wait=[w], on_update=[])
                    nop.bass_nofuse = True
                    out.append(nop)
                inst.sync_info = mybir.SyncInfo(
                    on_wait=[waits[-1]],
                    on_update=list(si.on_update) if si.on_update else [])
            out.append(inst)
        if changed:
            insts[:] = out


class _TileClockWaitSplit:
    _is_split_wrapper = True
    _real = _RealTileClockWait

    def __init__(self, *args, **kwargs):
        self._inner = _RealTileClockWait(*args, **kwargs)
        self._ordered = args[1] if len(args) > 1 else kwargs["ordered_instructions_by_block"]

    def __getattr__(self, k):
        return getattr(self._inner, k)

    def assign_waits(self, bb_name):
        r = self._inner.assign_waits(bb_name)
        _split_excess_waits(self._ordered)
        return r


_tile.TileClockWait = _TileClockWaitSplit

# ---------------------------------------------------------------------------
B, N, DIM, H, HD = 64, 197, 768, 12, 64
NCORES = 8
BL = B // NCORES            # 8 batches per core
NQ0, NQ1 = 127, 70          # keys split: keys 0..126 | keys 127..196
F32 = mybir.dt.float32
BF16 = mybir.dt.bfloat16
AF = mybir.ActivationFunctionType
ALU = mybir.AluOpType

_graph_cache = {}


def _build_graph():
    nc = bass.Bass()
    xT = nc.declare_dram_parameter("xT", [6, 128, BL * N], BF16, isOutput=False)
    wqkvT = nc.declare_dram_parameter("wqkvT", [DIM, 3 * DIM], BF16, isOutput=False)
    wprojT = nc.declare_dram_parameter("wprojT", [6, 128, DIM], BF16, isOutput=False)
    qkbias = nc.declare_dram_parameter("qkbias", [128, 12], F32, isOutput=False)
    expbT = nc.declare_dram_parameter("expbT", [NQ0, H * 2 * N], BF16, isOutput=False)
    onesb = nc.declare_dram_parameter("onesb", [128, H], BF16, isOutput=False)
    yT = nc.declare_dram_parameter("yT", [BL, DIM, N], BF16, isOutput=True)

    rstage = nc.dram_tensor("recip_stage", [BL, 2432], BF16)
    rstage_s = nc.dram_tensor("sums_stage", [BL, 2432], BF16)

    with nc.allow_low_precision(reason="bf16 compute validated: rel_err 4e-3 vs 2e-2 gate"), \
         TileContext(nc) as tc:
        with tc.tile_pool(name="const", bufs=1) as cpool, \
             tc.tile_pool(name="small", bufs=3) as spool:

            wqq = [cpool.tile([128, DIM], BF16, tag=f"wqq{c}", name=f"wqq{c}") for c in range(6)]
            wqk = [cpool.tile([128, DIM], BF16, tag=f"wqk{c}", name=f"wqk{c}") for c in range(6)]
            wqv = [cpool.tile([128, DIM], BF16, tag=f"wqv{c}", name=f"wqv{c}") for c in range(6)]
            wp = [cpool.tile([128, DIM], BF16, tag=f"wp{c}", name=f"wp{c}") for c in range(6)]
            qkb = cpool.tile([128, 12], F32, tag="qkb")
            ons = cpool.tile([128, H], BF16, tag="ons")
            expb = cpool.tile([NQ0, H * 2 * N], BF16, tag="expb")

            _xpool_cm = tc.tile_pool(name="xp", bufs=1)
            xpool = _xpool_cm.__enter__()
            xall = [xpool.tile([128, BL * N], BF16, tag=f"x{c}", name=f"x{c}") for c in range(6)]

            # ---- load schedule ------------------------------------------
            nc.sync.dma_start(out=qkb[:], in_=qkbias[:])
            nc.scalar.dma_start(out=ons[:], in_=onesb[:])
            # one DMA per tile (same-tile splits serialize in Tile);
            # x + wq-q are the critical set for the first matmuls, spread
            # over all three issuing rows (sync/scalar/gpsimd ~90GB/s each)
            engs = [nc.sync, nc.scalar, nc.gpsimd]
            _crit_loads = []
            for c in range(6):
                _crit_loads.append(engs[c % 2].dma_start(out=xall[c][:], in_=xT[c]))
            for c in range(6):
                _crit_loads.append(nc.gpsimd.dma_start(
                    out=wqq[c][:],
                    in_=bass.AP(wqkvT, c * 128 * 3 * DIM, [[3 * DIM, 128], [1, DIM]])))
            for c in range(6):
                engs[c % 2].dma_start(
                    out=wqk[c][:],
                    in_=bass.AP(wqkvT, c * 128 * 3 * DIM + DIM, [[3 * DIM, 128], [1, DIM]]))
            for c in range(6):
                engs[c % 2].dma_start(
                    out=wqv[c][:],
                    in_=bass.AP(wqkvT, c * 128 * 3 * DIM + 2 * DIM, [[3 * DIM, 128], [1, DIM]]))
            nc.gpsimd.dma_start(out=expb[:], in_=expbT[:])
            for c in range(6):
                nc.gpsimd.dma_start(out=wp[c][:], in_=wprojT[c])

            # ---- qk projection: qkTm[m] = [128, BL*197] bf16 ------------
            qkTm = [cpool.tile([128, BL * N], BF16, tag=f"qk{m}", name=f"qkTm{m}") for m in range(12)]
            with tc.tile_pool(name="ps_qk", bufs=8, space="PSUM") as pqk:
                for m in range(12):
                    pss = [pqk.tile([128, 2 * N], F32, tag="qkps", name=f"qkps{m}_{_}") for _ in range(4)]
                    wsrc = wqq if m < 6 else wqk
                    mc = m % 6
                    for c in range(6):
                        for bp in range(4):
                            nc.tensor.matmul(
                                pss[bp][:],
                                wsrc[c][:, 128 * mc:128 * (mc + 1)],
                                xall[c][:, bp * 2 * N:(bp + 1) * 2 * N],
                                start=(c == 0), stop=(c == 5))
                    sc = 0.125 if m < 6 else 1.0
                    for bp in range(4):
                        nc.vector.tensor_scalar(
                            out=qkTm[m][:, bp * 2 * N:(bp + 1) * 2 * N],
                            in0=pss[bp][:], scalar1=sc,
                            scalar2=qkb[:, m:m + 1], op0=ALU.mult, op1=ALU.add)

            # ---- v projection (no v_bias: folded to host) ---------------
            v_sb = [[cpool.tile([NQ0 if nch == 0 else NQ1, DIM], BF16,
                                tag=f"v{b}_{nch}", name=f"v{b}_{nch}") for nch in range(2)] for b in range(BL)]

            def emit_vproj(b, psum_pool, tag, shape):
                for nch in range(2):
                    nn_, nb = (NQ0, 0) if nch == 0 else (NQ1, NQ0)
                    ps = [psum_pool.tile(shape, F32, tag=tag,
                                         name=f"vps{b}_{nch}_{_}") for _ in range(2)]
                    for c in range(6):
                        for fh in range(2):
                            nc.tensor.matmul(
                                ps[fh][0:nn_, 0:384],
                                xall[c][:, b * N + nb:b * N + nb + nn_],
                                wqv[c][:, 384 * fh:384 * (fh + 1)],
                                start=(c == 0), stop=(c == 5))
                    for fh in range(2):
                        if fh == 0:
                            nc.scalar.activation(
                                out=v_sb[b][nch][0:nn_, 0:384],
                                in_=ps[0][0:nn_, 0:384], func=AF.Copy)
                        else:
                            nc.vector.tensor_copy(
                                out=v_sb[b][nch][0:nn_, 384:768],
                                in_=ps[1][0:nn_, 0:384])

            _pv_cm = tc.tile_pool(name="ps_v", bufs=4, space="PSUM")
            pv = _pv_cm.__enter__()
            for b in range(6):
                emit_vproj(b, pv, "vps", [NQ0, 2 * N])
            _pv_cm.__exit__(None, None, None)

            _apool_cm = tc.tile_pool(name="ap", bufs=1)
            apool = _apool_cm.__enter__()

            outT = [[apool.tile([128, N], BF16, tag=f"o{cc}", name=f"outT{b}_{cc}", bufs=3) for cc in range(6)]
                    for b in range(BL)]
            ups = {}
            rbcs = {}
            ehms = {}

            with tc.tile_pool(name="ps_at", bufs=2, space="PSUM") as pat, \
                 tc.tile_pool(name="ps_av", bufs=1, space="PSUM") as pav, \
                 tc.tile_pool(name="ps_pj", bufs=1, space="PSUM") as ppjp, \
                 tc.tile_pool(name="ps_ss", bufs=2, space="PSUM") as pssp:
                ehms = {}

                def emit_qk_pair(b, hp):
                    mq, mk = hp, 6 + hp
                    psh = pat.tile([NQ0, 1024], F32, tag="atps",
                                   name=f"atps{b}_{hp}")
                    ehp = apool.tile([NQ0, 4 * N], BF16, tag="ehs",
                                     name=f"eh{b}_{hp}", bufs=2)
                    for hh in range(2):
                        rb = hh * 64
                        q0 = hh * 512
                        nc.tensor.matmul(
                            psh[0:NQ0, q0:q0 + N],
                            qkTm[mk][rb:rb + 64, b * N:b * N + NQ0],
                            qkTm[mq][rb:rb + 64, b * N:b * N + N],
                            start=True, stop=True)
                        nc.tensor.matmul(
                            psh[0:NQ1, q0 + N:q0 + 2 * N],
                            qkTm[mk][rb:rb + 64, b * N + NQ0:b * N + N],
                            qkTm[mq][rb:rb + 64, b * N:b * N + N],
                            start=True, stop=True)
                    nc.scalar.activation(
                        out=bass.AP(ehp[:].tensor, 0, [[4 * N, NQ0], [2 * N, 2], [1, 2 * N]]),
                        in_=bass.AP(psh[:].tensor, 0, [[1024, NQ0], [512, 2], [1, 2 * N]]),
                        func=AF.Exp)
                    ehm = ehms[b][hp]
                    nc.vector.tensor_tensor(
                        out=ehm[:], in0=ehp[:],
                        in1=expb[0:NQ0, hp * 4 * N:(hp + 1) * 4 * N],
                        op=ALU.mult)

                for slot in range(BL + 1):
                    if slot == 0:
                        ehms[0] = [apool.tile([NQ0, 4 * N], BF16, tag=f"em{hp}",
                                              name=f"em0_{hp}", bufs=2) for hp in range(6)]
                        for hp in range(6):
                            emit_qk_pair(0, hp)
                    # ---- norm-apply for batch slot-1 (gpsimd) ----
                    bm1 = slot - 1
                    if 0 <= bm1 < BL:
                        rbc = rbcs.pop(bm1)
                        up = ups.pop(bm1)
                        for cc in range(6):
                            nc.gpsimd.tensor_tensor(
                                out=outT[bm1][cc][:], in0=up[cc][:],
                                in1=rbc[:, cc * N:(cc + 1) * N], op=ALU.mult)

                    b = slot if slot < BL else None
                    bnx = slot + 1 if slot + 1 < BL else None
                    bm2 = slot - 2
                    # proj schedule: lag 2 for b<=4, lag 1 for b>=5 (shorter
                    # pipeline drain); maps step index -> (batch, pair)
                    proj_steps = {}
                    if 2 <= slot <= 6:
                        for pr in range(3):
                            proj_steps[pr] = (slot - 2, pr)
                    if 6 <= slot <= 8:
                        for pr in range(3):
                            proj_steps[3 + pr] = (slot - 1, pr)
                    if bnx is not None:
                        ehms[bnx] = [apool.tile([NQ0, 4 * N], BF16, tag=f"em{hp}",
                                                name=f"em{bnx}_{hp}", bufs=2) for hp in range(6)]
                    if b is not None:
                        ehm_b = ehms.pop(b)
                        srow = apool.tile([1, 2432], BF16, tag="srow", bufs=2,
                                          name=f"srow{b}")
                        up = [apool.tile([128, N], BF16, tag=f"up{cc}",
                                         name=f"up{b}_{cc}", bufs=2) for cc in range(6)]
                        ups[b] = up

                    def emit_proj_pair(pb, pr):
                        ppx = ppjp.tile([128, 2 * N], F32, tag="pjps",
                                        name=f"pj{pb}_{pr}")
                        for half in range(2):
                            mp = 2 * pr + half
                            for c in range(6):
                                nc.tensor.matmul(
                                    ppx[:, half * N:(half + 1) * N],
                                    wp[c][:, 128 * mp:128 * (mp + 1)],
                                    outT[pb][c][:], start=(c == 0), stop=(c == 5))
                        ysb = spool.tile([128, 2 * N], BF16, tag="ysb", name=f"ysb{pb}_{pr}")
                        nc.scalar.activation(out=ysb[:], in_=ppx[:], func=AF.Copy)
                        nc.sync.dma_start(
                            out=bass.AP(yT, pb * DIM * N + 2 * pr * 128 * N,
                                        [[N, 128], [128 * N, 2], [1, N]]),
                            in_=ysb[:])

                    def emit_av(hp):
                        pp = pav.tile([128, N], F32, tag="avps", name=f"avps{b}_{hp}")
                        for hh in range(2):
                            h = 2 * hp + hh
                            rb = hh * 64
                            e0 = hh * 2 * N
                            nc.tensor.matmul(pp[rb:rb + 64, :],
                                             v_sb[b][0][:, h * 64:(h + 1) * 64],
                                             ehm_b[hp][0:NQ0, e0:e0 + N],
                                             start=True, stop=False,
                                             tile_position=(0, rb))
                            nc.tensor.matmul(pp[rb:rb + 64, :],
                                             v_sb[b][1][:, h * 64:(h + 1) * 64],
                                             ehm_b[hp][0:NQ1, e0 + N:e0 + 2 * N],
                                             start=False, stop=True,
                                             tile_position=(0, rb))
                        nc.vector.tensor_copy(out=up[hp][:], in_=pp[:])

                    def emit_sums(hp):
                        pss = pssp.tile([1, 2 * N], F32, tag="smps", name=f"smps{b}_{hp}")
                        for hh in range(2):
                            e0 = hh * 2 * N
                            nc.tensor.matmul(pss[0:1, hh * N:(hh + 1) * N],
                                             ons[0:NQ0, 0:1],
                                             ehm_b[hp][0:NQ0, e0:e0 + N],
                                             start=True, stop=False)
                            nc.tensor.matmul(pss[0:1, hh * N:(hh + 1) * N],
                                             ons[0:NQ1, 0:1],
                                             ehm_b[hp][0:NQ1, e0 + N:e0 + 2 * N],
                                             start=False, stop=True)
                        if hp % 2 == 0:
                            nc.scalar.activation(
                                out=srow[0:1, hp * 2 * N:(hp + 1) * 2 * N],
                                in_=pss[0:1, :], func=AF.Copy)
                        else:
                            nc.vector.tensor_copy(
                                out=srow[0:1, hp * 2 * N:(hp + 1) * 2 * N], in_=pss[0:1, :])

                    # interleaved PE emission: QK pairs spaced by AV/proj;
                    # sums lag their AV by one step so the srow copy hides
                    for step in range(6):
                        if bnx is not None:
                            emit_qk_pair(bnx, step)
                        if slot in (0, 1) and step == 0:
                            emit_vproj(6 + slot, pat, "atps", [NQ0, 1024])
                        if b is not None:
                            emit_av(step)
                            if step > 0:
                                emit_sums(step - 1)
                        if step in proj_steps:
                            emit_proj_pair(*proj_steps[step])
                    if b is not None:
                        emit_sums(5)

                    # ---- normalization chain for batch slot ----
                    if b is not None:
                        swr = nc.sync.dma_start(out=rstage_s[b:b + 1, :], in_=srow[0:1, :])
                        s128 = apool.tile([128, 19], BF16, tag="s128", bufs=2)
                        srd = nc.sync.dma_start(
                            out=s128[:], in_=bass.AP(rstage_s, b * 2432, [[19, 128], [1, 19]]))
                        add_dep_helper(srd.ins, swr.ins, sync=True, reason="sums staging")
                        r128 = apool.tile([128, 19], BF16, tag="r128", bufs=2)
                        nc.vector.reciprocal(out=r128[:], in_=s128[:])
                        rwr = nc.sync.dma_start(
                            out=bass.AP(rstage, b * 2432, [[19, 128], [1, 19]]), in_=r128[:])
                        rbc = apool.tile([128, 6 * N], BF16, tag="rbc", bufs=2,
                                         name=f"rbc{b}")
                        rbcs[b] = rbc
                        rrd = nc.sync.dma_start(
                            out=bass.AP(rbc[:].tensor, 0, [[6 * N, 64], [N, 6], [1, N]]),
                            in_=bass.AP(rstage, b * 2432, [[0, 64], [2 * N, 6], [1, N]]))
                        rrd2 = nc.sync.dma_start(
                            out=bass.AP(rbc[:].tensor, 64 * 6 * N, [[6 * N, 64], [N, 6], [1, N]]),
                            in_=bass.AP(rstage, b * 2432 + N, [[0, 64], [2 * N, 6], [1, N]]))
                        add_dep_helper(rrd.ins, rwr.ins, sync=True, reason="recip staging")
                        add_dep_helper(rrd2.ins, rwr.ins, sync=True, reason="recip staging")
            _apool_cm.__exit__(None, None, None)
            _xpool_cm.__exit__(None, None, None)
    return nc


def _prep_inputs(x, Wqkv, q_bias, v_bias, rel_table, Wproj, bproj, rel_index):
    bf = ml_dtypes.bfloat16
    xs = np.asarray(x).astype(bf)                         # [B, N, DIM]
    xT = np.ascontiguousarray(xs.transpose(2, 0, 1))      # [DIM, B, N]
    wqkvT = np.ascontiguousarray(np.asarray(Wqkv).T).astype(bf)
    wprojT = np.ascontiguousarray(np.asarray(Wproj).T).astype(bf).reshape(6, 128, DIM)
    qs = np.concatenate([np.asarray(q_bias) * (HD ** -0.5), np.zeros(DIM, np.float32)])
    qkbias = np.ascontiguousarray(qs.reshape(12, 128).T).astype(np.float32)
    bias = np.asarray(rel_table)[np.asarray(rel_index)]   # [N(q), N(k), H]
    eb = np.exp(bias.transpose(2, 0, 1).astype(np.float32))  # [H, q, k]
    expbT = np.zeros((NQ0, H * 2 * N), dtype=np.float32)
    ebT = eb.transpose(0, 2, 1)                           # [H, k, q]
    for h in range(H):
        expbT[0:NQ0, h * 2 * N:h * 2 * N + N] = ebT[h, 0:NQ0, :]
        expbT[0:NQ1, h * 2 * N + N:(h + 1) * 2 * N] = ebT[h, NQ0:N, :]
    expbT = expbT.astype(bf)
    onesb = np.ones((128, H), dtype=bf)
    # exact host fold: y += bproj + v_bias @ Wproj.T
    ybias = (np.asarray(bproj) +
             np.asarray(v_bias).astype(np.float64) @ np.asarray(Wproj).astype(np.float64).T
             ).astype(np.float32)
    return xT, wqkvT, wprojT, qkbias, expbT, onesb, ybias


def run_sharded(inputs, trace=False):
    nc = _graph_cache.get("nc")
    if nc is None:
        nc = _build_graph()
        _graph_cache["nc"] = nc
    xT, wqkvT, wprojT, qkbias, expbT, onesb, ybias = _prep_inputs(**inputs)
    in_maps = []
    for i in range(NCORES):
        in_maps.append({
            "xT": np.ascontiguousarray(
                xT[:, i * BL:(i + 1) * BL, :].reshape(6, 128, BL * N)),
            "wqkvT": wqkvT, "wprojT": wprojT, "qkbias": qkbias,
            "expbT": expbT, "onesb": onesb,
        })
    res = run_bass_kernel_spmd(nc, in_maps, list(range(NCORES)), trace=trace)
    outs = []
    for i in range(NCORES):
        ytc = np.asarray(res.results[i]["yT"]).astype(np.float32)  # [BL, DIM, N]
        outs.append(ytc.transpose(0, 2, 1))             # [BL, N, DIM]
    y = np.concatenate(outs, axis=0).astype(np.float32) + ybias
    return y, res


def kernel(**inputs) -> np.ndarray:
    y, _ = run_sharded(inputs, trace=False)
    return y


# revision 25
# speedup vs baseline: 1.0024x; 1.0002x over previous
"""Trainium2 Bass kernel: 12-head attention with relative position bias.

Reference computation (B=64, N=197, DIM=768, H=12, HD=64):
    qkv = x @ Wqkv.T + [q_bias, 0, v_bias]
    q, k, v = split(qkv); q *= HD**-0.5
    attn = softmax(q @ k.T + rel_table[rel_index].T)   # bias per head
    y = (attn @ v).reshape @ Wproj.T + bproj

Strategy: pure data-parallel over batch (8 batches per NeuronCore x 8 cores,
no collectives), bf16 matmuls with fp32 PSUM. Attention is computed
transposed (attnT = k q^T) so attn @ v needs no transpose; softmax has no
max-subtraction (|logits| < 3); per-query sums via ones-vector matmuls;
normalization applied post-AV via a DRAM-staged partition broadcast of
reciprocal sums. Exact host-side algebra folds remove device work:
  - exp(rel_bias) precomputed on host, loaded as one dense tile
  - v_bias and bproj leave the device entirely:
      y_device = (attn @ (x@Wv)) / sums @ Wp.T
      y = y_device + (bproj + v_bias @ Wp.T)     [host, exact]
The kernel is software-pipelined per batch slot s:
  norm-apply(s-1) on gpsimd | QK+exp+mult(s+1) | proj(s-2) interleaved
  with AV+sums(s) | norm chain(s) on sync/vector.
Leftover v-projections (batches 6,7) fill slots 0/1 so the PE stays busy
and the HAM clock gate keeps the PE at 2.4 GHz.
"""
import sys
sys.path.insert(0, '/opt/trn_rl_repo')
import itertools
import numpy as np
import ml_dtypes

import concourse.bass as bass
import concourse.mybir as mybir
from concourse import tile as _tile
from concourse.tile import TileContext, add_dep_helper
from concourse.vector_clock import ScopedClock
from concourse.bass_utils import run_bass_kernel_spmd

# ---------------------------------------------------------------------------
# Patches for this toolchain's one-sync-wait-per-instruction limit.
# ---------------------------------------------------------------------------
_counter = itertools.count()


def _drain_and_barrier_split(self, tick_clock, wait_clock):
    vc = tick_clock.global_clock
    for proc in range(len(vc)):
        t = vc[proc]
        if t > 0:
            sc = ScopedClock()
            sc.require_at_least(None, proc, t)
            nop_inst = self.nc.sync.nop(nofuse=True, hint="drain_split")
            wait_clock.add_sem_waits(nop_inst.ins, sc)
    self.nc.sync.drain()
    self.nc.all_engine_barrier()
    popped = self.nc._tile_sem_poison_stack.pop()
    assert popped is self._sem_poison
    self.nc.clear_and_free_semaphores(list(self.sems.allocated().values()))
    self.nc.all_engine_barrier()


_tile.TileContext._drain_and_barrier = _drain_and_barrier_split

_RealTileClockWait = _tile.TileClockWait
if getattr(_RealTileClockWait, "_is_split_wrapper", False):  # re-import safety
    _RealTileClockWait = _RealTileClockWait._real


def _split_excess_waits(ordered):
    for bb_name, insts in ordered.items():
        out = []
        changed = False
        for inst in insts:
            si = inst.sync_info
            waits = list(si.on_wait) if si is not None and si.on_wait else []
            if len(waits) > 1:
                changed = True
                for w in waits[:-1]:
                    nop = mybir.InstNoOp(
                        name=f"waitsplit_{next(_counter)}", engine=inst.engine)
                    nop.sync_info = mybir.SyncInfo(on_wait=[w], on_update=[])
                    nop.bass_nofuse = True
                    out.append(nop)
                inst.sync_info = mybir.SyncInfo(
                    on_wait=[waits[-1]],
                    on_update=list(si.on_update) if si.on_update else [])
            out.append(inst)
        if changed:
            insts[:] = out


class _TileClockWaitSplit:
    _is_split_wrapper = True
    _real = _RealTileClockWait

    def __init__(self, *args, **kwargs):
        self._inner = _RealTileClockWait(*args, **kwargs)
        self._ordered = args[1] if len(args) > 1 else kwargs["ordered_instructions_by_block"]

    def __getattr__(self, k):
        return getattr(self._inner, k)

    def assign_waits(self, bb_name):
        r = self._inner.assign_waits(bb_name)
        _split_excess_waits(self._ordered)
        return r


_tile.TileClockWait = _TileClockWaitSplit

# ---------------------------------------------------------------------------
B, N, DIM, H, HD = 64, 197, 768, 12, 64
NCORES = 8
BL = B // NCORES            # 8 batches per core
NQ0, NQ1 = 127, 70          # keys split: keys 0..126 | keys 127..196
F32 = mybir.dt.float32
BF16 = mybir.dt.bfloat16
AF = mybir.ActivationFunctionType
ALU = mybir.AluOpType

_graph_cache = {}


def _build_graph():
    nc = bass.Bass()
    xT = nc.declare_dram_parameter("xT", [6, 128, BL * N], BF16, isOutput=False)
    wqkvT = nc.declare_dram_parameter("wqkvT", [DIM, 3 * DIM], BF16, isOutput=False)
    wprojT = nc.declare_dram_parameter("wprojT", [6, 128, DIM], BF16, isOutput=False)
    qkbias = nc.declare_dram_parameter("qkbias", [128, 12], F32, isOutput=False)
    expbT = nc.declare_dram_parameter("expbT", [NQ0, H * 2 * N], BF16, isOutput=False)
    onesb = nc.declare_dram_parameter("onesb", [128, H], BF16, isOutput=False)
    yT = nc.declare_dram_parameter("yT", [BL, DIM, N], BF16, isOutput=True)

    rstage = nc.dram_tensor("recip_stage", [BL, 2432], BF16)
    rstage_s = nc.dram_tensor("sums_stage", [BL, 2432], BF16)

    with nc.allow_low_precision(reason="bf16 compute validated: rel_err 4e-3 vs 2e-2 gate"), \
         TileContext(nc) as tc:
        with tc.tile_pool(name="const", bufs=1) as cpool, \
             tc.tile_pool(name="small", bufs=3) as spool:

            wqq = [cpool.tile([128, DIM], BF16, tag=f"wqq{c}", name=f"wqq{c}") for c in range(6)]
            wqk = [cpool.tile([128, DIM], BF16, tag=f"wqk{c}", name=f"wqk{c}") for c in range(6)]
            wqv = [cpool.tile([128, DIM], BF16, tag=f"wqv{c}", name=f"wqv{c}") for c in range(6)]
            wp = [cpool.tile([128, DIM], BF16, tag=f"wp{c}", name=f"wp{c}") for c in range(6)]
            qkb = cpool.tile([128, 12], F32, tag="qkb")
            ons = cpool.tile([128, H], BF16, tag="ons")
            expb = cpool.tile([NQ0, H * 2 * N], BF16, tag="expb")

            _xpool_cm = tc.tile_pool(name="xp", bufs=1)
            xpool = _xpool_cm.__enter__()
            xall = [xpool.tile([128, BL * N], BF16, tag=f"x{c}", name=f"x{c}") for c in range(6)]

            # ---- load schedule ------------------------------------------
            nc.sync.dma_start(out=qkb[:], in_=qkbias[:])
            nc.scalar.dma_start(out=ons[:], in_=onesb[:])
            # one DMA per tile (same-tile splits serialize in Tile);
            # x + wq-q are the critical set for the first matmuls, spread
            # over all three issuing rows (sync/scalar/gpsimd ~90GB/s each)
            engs = [nc.sync, nc.scalar, nc.gpsimd]
            _crit_loads = []
            for c in range(6):
                _crit_loads.append(engs[c % 2].dma_start(out=xall[c][:], in_=xT[c]))
            for c in range(6):
                _crit_loads.append(nc.gpsimd.dma_start(
                    out=wqq[c][:],
                    in_=bass.AP(wqkvT, c * 128 * 3 * DIM, [[3 * DIM, 128], [1, DIM]])))
            for c in range(6):
                engs[c % 2].dma_start(
                    out=wqk[c][:],
                    in_=bass.AP(wqkvT, c * 128 * 3 * DIM + DIM, [[3 * DIM, 128], [1, DIM]]))
            for c in range(6):
                engs[c % 2].dma_start(
                    out=wqv[c][:],
                    in_=bass.AP(wqkvT, c * 128 * 3 * DIM + 2 * DIM, [[3 * DIM, 128], [1, DIM]]))
            nc.gpsimd.dma_start(out=expb[:], in_=expbT[:])
            for c in range(6):
                nc.gpsimd.dma_start(out=wp[c][:], in_=wprojT[c])

            # ---- qk projection: qkTm[m] = [128, BL*197] bf16 ------------
            qkTm = [cpool.tile([128, BL * N], BF16, tag=f"qk{m}", name=f"qkTm{m}") for m in range(12)]
            with tc.tile_pool(name="ps_qk", bufs=8, space="PSUM") as pqk:
                for m in range(12):
                    pss = [pqk.tile([128, 2 * N], F32, tag="qkps", name=f"qkps{m}_{_}") for _ in range(4)]
                    wsrc = wqq if m < 6 else wqk
                    mc = m % 6
                    for c in range(6):
                        for bp in range(4):
                            nc.tensor.matmul(
                                pss[bp][:],
                                wsrc[c][:, 128 * mc:128 * (mc + 1)],
                                xall[c][:, bp * 2 * N:(bp + 1) * 2 * N],
                                start=(c == 0), stop=(c == 5))
                    sc = 0.125 if m < 6 else 1.0
                    for bp in range(4):
                        nc.vector.tensor_scalar(
                            out=qkTm[m][:, bp * 2 * N:(bp + 1) * 2 * N],
                            in0=pss[bp][:], scalar1=sc,
                            scalar2=qkb[:, m:m + 1], op0=ALU.mult, op1=ALU.add)

            # ---- v projection (no v_bias: folded to host) ---------------
            v_sb = [[cpool.tile([NQ0 if nch == 0 else NQ1, DIM], BF16,
                                tag=f"v{b}_{nch}", name=f"v{b}_{nch}") for nch in range(2)] for b in range(BL)]

            def emit_vproj(b, psum_pool, tag, shape):
                for nch in range(2):
                    nn_, nb = (NQ0, 0) if nch == 0 else (NQ1, NQ0)
                    ps = [psum_pool.tile(shape, F32, tag=tag,
                                         name=f"vps{b}_{nch}_{_}") for _ in range(2)]
                    for c in range(6):
                        for fh in range(2):
                            nc.tensor.matmul(
                                ps[fh][0:nn_, 0:384],
                                xall[c][:, b * N + nb:b * N + nb + nn_],
                                wqv[c][:, 384 * fh:384 * (fh + 1)],
                                start=(c == 0), stop=(c == 5))
                    for fh in range(2):
                        if fh == 0:
                            nc.scalar.activation(
                                out=v_sb[b][nch][0:nn_, 0:384],
                                in_=ps[0][0:nn_, 0:384], func=AF.Copy)
                        else:
                            nc.vector.tensor_copy(
                                out=v_sb[b][nch][0:nn_, 384:768],
                                in_=ps[1][0:nn_, 0:384])

            _pv_cm = tc.tile_pool(name="ps_v", bufs=4, space="PSUM")
            pv = _pv_cm.__enter__()
            for b in range(6):
                emit_vproj(b, pv, "vps", [NQ0, 2 * N])
            _pv_cm.__exit__(None, None, None)

            _apool_cm = tc.tile_pool(name="ap", bufs=1)
            apool = _apool_cm.__enter__()

            outT = [[apool.tile([128, N], BF16, tag=f"o{cc}", name=f"outT{b}_{cc}", bufs=3) for cc in range(6)]
                    for b in range(BL)]
            ups = {}
            rbcs = {}
            ehms = {}

            with tc.tile_pool(name="ps_at", bufs=2, space="PSUM") as pat, \
                 tc.tile_pool(name="ps_av", bufs=1, space="PSUM") as pav, \
                 tc.tile_pool(name="ps_pj", bufs=1, space="PSUM") as ppjp, \
                 tc.tile_pool(name="ps_ss", bufs=2, space="PSUM") as pssp:
                ehms = {}

                def emit_qk_pair(b, hp):
                    mq, mk = hp, 6 + hp
                    psh = pat.tile([NQ0, 1024], F32, tag="atps",
                                   name=f"atps{b}_{hp}")
                    ehp = apool.tile([NQ0, 4 * N], BF16, tag="ehs",
                                     name=f"eh{b}_{hp}", bufs=2)
                    for hh in range(2):
                        rb = hh * 64
                        q0 = hh * 512
                        nc.tensor.matmul(
                            psh[0:NQ0, q0:q0 + N],
                            qkTm[mk][rb:rb + 64, b * N:b * N + NQ0],
                            qkTm[mq][rb:rb + 64, b * N:b * N + N],
                            start=True, stop=True)
                        nc.tensor.matmul(
                            psh[0:NQ1, q0 + N:q0 + 2 * N],
                            qkTm[mk][rb:rb + 64, b * N + NQ0:b * N + N],
                            qkTm[mq][rb:rb + 64, b * N:b * N + N],
                            start=True, stop=True)
                    nc.scalar.activation(
                        out=bass.AP(ehp[:].tensor, 0, [[4 * N, NQ0], [2 * N, 2], [1, 2 * N]]),
                        in_=bass.AP(psh[:].tensor, 0, [[1024, NQ0], [512, 2], [1, 2 * N]]),
                        func=AF.Exp)
                    ehm = ehms[b][hp]
                    nc.vector.tensor_tensor(
                        out=ehm[:], in0=ehp[:],
                        in1=expb[0:NQ0, hp * 4 * N:(hp + 1) * 4 * N],
                        op=ALU.mult)

                for slot in range(BL + 1):
                    if slot == 0:
                        ehms[0] = [apool.tile([NQ0, 4 * N], BF16, tag=f"em{hp}",
                                              name=f"em0_{hp}", bufs=2) for hp in range(6)]
                        for hp in range(6):
                            emit_qk_pair(0, hp)
                    # ---- norm-apply for batch slot-1 (gpsimd) ----
                    bm1 = slot - 1
                    if 0 <= bm1 < BL:
                        rbc = rbcs.pop(bm1)
                        up = ups.pop(bm1)
                        for cc in range(6):
                            nc.gpsimd.tensor_tensor(
                                out=outT[bm1][cc][:], in0=up[cc][:],
                                in1=rbc[:, cc * N:(cc + 1) * N], op=ALU.mult)

                    b = slot if slot < BL else None
                    bnx = slot + 1 if slot + 1 < BL else None
                    bm2 = slot - 2
                    # proj schedule: lag 2 for b<=4, lag 1 for b>=5 (shorter
                    # pipeline drain); maps step index -> (batch, pair)
                    proj_steps = {}
                    if 2 <= slot <= 6:
                        for pr in range(3):
                            proj_steps[2 * pr + (0 if slot < 6 else 0)] = (slot - 2, pr)
                    if 6 <= slot <= 8:
                        for pr in range(3):
                            proj_steps[2 * pr + 1] = (slot - 1, pr)
                    if bnx is not None:
                        ehms[bnx] = [apool.tile([NQ0, 4 * N], BF16, tag=f"em{hp}",
                                                name=f"em{bnx}_{hp}", bufs=2) for hp in range(6)]
                    if b is not None:
                        ehm_b = ehms.pop(b)
                        srow = apool.tile([1, 2432], BF16, tag="srow", bufs=2,
                                          name=f"srow{b}")
                        up = [apool.tile([128, N], BF16, tag=f"up{cc}",
                                         name=f"up{b}_{cc}", bufs=2) for cc in range(6)]
                        ups[b] = up

                    def emit_proj_pair(pb, pr):
                        ppx = ppjp.tile([128, 2 * N], F32, tag="pjps",
                                        name=f"pj{pb}_{pr}")
                        for half in range(2):
                            mp = 2 * pr + half
                            for c in range(6):
                                nc.tensor.matmul(
                                    ppx[:, half * N:(half + 1) * N],
                                    wp[c][:, 128 * mp:128 * (mp + 1)],
                                    outT[pb][c][:], start=(c == 0), stop=(c == 5))
                        ysb = spool.tile([128, 2 * N], BF16, tag="ysb", name=f"ysb{pb}_{pr}")
                        nc.scalar.activation(out=ysb[:], in_=ppx[:], func=AF.Copy)
                        nc.sync.dma_start(
                            out=bass.AP(yT, pb * DIM * N + 2 * pr * 128 * N,
                                        [[N, 128], [128 * N, 2], [1, N]]),
                            in_=ysb[:])

                    def emit_av(hp):
                        pp = pav.tile([128, N], F32, tag="avps", name=f"avps{b}_{hp}")
                        for hh in range(2):
                            h = 2 * hp + hh
                            rb = hh * 64
                            e0 = hh * 2 * N
                            nc.tensor.matmul(pp[rb:rb + 64, :],
                                             v_sb[b][0][:, h * 64:(h + 1) * 64],
                                             ehm_b[hp][0:NQ0, e0:e0 + N],
                                             start=True, stop=False,
                                             tile_position=(0, rb))
                            nc.tensor.matmul(pp[rb:rb + 64, :],
                                             v_sb[b][1][:, h * 64:(h + 1) * 64],
                                             ehm_b[hp][0:NQ1, e0 + N:e0 + 2 * N],
                                             start=False, stop=True,
                                             tile_position=(0, rb))
                        nc.vector.tensor_copy(out=up[hp][:], in_=pp[:])

                    def emit_sums(hp):
                        pss = pssp.tile([1, 2 * N], F32, tag="smps", name=f"smps{b}_{hp}")
                        for hh in range(2):
                            e0 = hh * 2 * N
                            nc.tensor.matmul(pss[0:1, hh * N:(hh + 1) * N],
                                             ons[0:NQ0, 0:1],
                                             ehm_b[hp][0:NQ0, e0:e0 + N],
                                             start=True, stop=False)
                            nc.tensor.matmul(pss[0:1, hh * N:(hh + 1) * N],
                                             ons[0:NQ1, 0:1],
                                             ehm_b[hp][0:NQ1, e0 + N:e0 + 2 * N],
                                             start=False, stop=True)
                        if hp % 2 == 0:
                            nc.scalar.activation(
                                out=srow[0:1, hp * 2 * N:(hp + 1) * 2 * N],
                                in_=pss[0:1, :], func=AF.Copy)
                        else:
                            nc.vector.tensor_copy(
                                out=srow[0:1, hp * 2 * N:(hp + 1) * 2 * N], in_=pss[0:1, :])

                    # interleaved PE emission: QK pairs spaced by AV/proj;
                    # sums lag their AV by one step so the srow copy hides
                    for step in range(6):
                        if bnx is not None:
                            emit_qk_pair(bnx, step)
                        if slot in (0, 1) and step == 0:
                            emit_vproj(6 + slot, pat, "atps", [NQ0, 1024])
                        if b is not None:
                            emit_av(step)
                            if step > 0:
                                emit_sums(step - 1)
                        if step in proj_steps:
                            emit_proj_pair(*proj_steps[step])
                    if b is not None:
                        emit_sums(5)

                    # ---- normalization chain for batch slot ----
                    if b is not None:
                        swr = nc.sync.dma_start(out=rstage_s[b:b + 1, :], in_=srow[0:1, :])
                        s128 = apool.tile([128, 19], BF16, tag="s128", bufs=2)
                        srd = nc.sync.dma_start(
                            out=s128[:], in_=bass.AP(rstage_s, b * 2432, [[19, 128], [1, 19]]))
                        add_dep_helper(srd.ins, swr.ins, sync=True, reason="sums staging")
                        r128 = apool.tile([128, 19], BF16, tag="r128", bufs=2)
                        nc.vector.reciprocal(out=r128[:], in_=s128[:])
                        rwr = nc.sync.dma_start(
                            out=bass.AP(rstage, b * 2432, [[19, 128], [1, 19]]), in_=r128[:])
                        rbc = apool.tile([128, 6 * N], BF16, tag="rbc", bufs=2,
                                         name=f"rbc{b}")
                        rbcs[b] = rbc
                        rrd = nc.sync.dma_start(
                            out=bass.AP(rbc[:].tensor, 0, [[6 * N, 64], [N, 6], [1, N]]),
                            in_=bass.AP(rstage, b * 2432, [[0, 64], [2 * N, 6], [1, N]]))
                        rrd2 = nc.sync.dma_start(
                            out=bass.AP(rbc[:].tensor, 64 * 6 * N, [[6 * N, 64], [N, 6], [1, N]]),
                            in_=bass.AP(rstage, b * 2432 + N, [[0, 64], [2 * N, 6], [1, N]]))
                        add_dep_helper(rrd.ins, rwr.ins, sync=True, reason="recip staging")
                        add_dep_helper(rrd2.ins, rwr.ins, sync=True, reason="recip staging")
            _apool_cm.__exit__(None, None, None)
            _xpool_cm.__exit__(None, None, None)
    return nc


def _prep_inputs(x, Wqkv, q_bias, v_bias, rel_table, Wproj, bproj, rel_index):
    bf = ml_dtypes.bfloat16
    xs = np.asarray(x).astype(bf)                         # [B, N, DIM]
    xT = np.ascontiguousarray(xs.transpose(2, 0, 1))      # [DIM, B, N]
    wqkvT = np.ascontiguousarray(np.asarray(Wqkv).T).astype(bf)
    wprojT = np.ascontiguousarray(np.asarray(Wproj).T).astype(bf).reshape(6, 128, DIM)
    qs = np.concatenate([np.asarray(q_bias) * (HD ** -0.5), np.zeros(DIM, np.float32)])
    qkbias = np.ascontiguousarray(qs.reshape(12, 128).T).astype(np.float32)
    bias = np.asarray(rel_table)[np.asarray(rel_index)]   # [N(q), N(k), H]
    eb = np.exp(bias.transpose(2, 0, 1).astype(np.float32))  # [H, q, k]
    expbT = np.zeros((NQ0, H * 2 * N), dtype=np.float32)
    ebT = eb.transpose(0, 2, 1)                           # [H, k, q]
    for h in range(H):
        expbT[0:NQ0, h * 2 * N:h * 2 * N + N] = ebT[h, 0:NQ0, :]
        expbT[0:NQ1, h * 2 * N + N:(h + 1) * 2 * N] = ebT[h, NQ0:N, :]
    expbT = expbT.astype(bf)
    onesb = np.ones((128, H), dtype=bf)
    # exact host fold: y += bproj + v_bias @ Wproj.T
    ybias = (np.asarray(bproj) +
             np.asarray(v_bias).astype(np.float64) @ np.asarray(Wproj).astype(np.float64).T
             ).astype(np.float32)
    return xT, wqkvT, wprojT, qkbias, expbT, onesb, ybias


def run_sharded(inputs, trace=False):
    nc = _graph_cache.get("nc")
    if nc is None:
        nc = _build_graph()
        _graph_cache["nc"] = nc
    xT, wqkvT, wprojT, qkbias, expbT, onesb, ybias = _prep_inputs(**inputs)
    in_maps = []
    for i in range(NCORES):
        in_maps.append({
            "xT": np.ascontiguousarray(
                xT[:, i * BL:(i + 1) * BL, :].reshape(6, 128, BL * N)),
            "wqkvT": wqkvT, "wprojT": wprojT, "qkbias": qkbias,
            "expbT": expbT, "onesb": onesb,
        })
    res = run_bass_kernel_spmd(nc, in_maps, list(range(NCORES)), trace=trace)
    outs = []
    for i in range(NCORES):
        ytc = np.asarray(res.results[i]["yT"]).astype(np.float32)  # [BL, DIM, N]
        outs.append(ytc.transpose(0, 2, 1))             # [BL, N, DIM]
    y = np.concatenate(outs, axis=0).astype(np.float32) + ybias
    return y, res


def kernel(**inputs) -> np.ndarray:
    y, _ = run_sharded(inputs, trace=False)
    return y
